# revision 29
# baseline (speedup 1.0000x reference)
"""Domain-specific BatchNorm (training mode) Trainium2 Bass kernel, v3.

Feature-sharded, collective-free: each of 8 cores owns 128 of 1024
features for ALL rows, so segment stats are core-local. Host stable-sorts
rows by domain and ships x transposed ([128 feat, N rows], fp16); each
domain is a contiguous column range baked into the program (built after
seeing y). Stats jobs (s1 sums, s2 square-sums) are list-scheduled over
DVE/ACT/Pool ordered by load-chunk arrival; scale/offset are computed in
two half-batches so early domains apply + store while late chunks still
load. Apply is one tensor_scalar (x*scale+off, per-partition fp16
scalars) per domain. No PE, no PSUM, no collective.
"""

import os
import sys

import numpy as np

for _p in ("/opt/trn_rl_repo", "/root/.axon_site/_ro/trn_rl_repo"):
    if os.path.isdir(_p) and _p not in sys.path:
        sys.path.insert(0, _p)

import concourse.bass as bass
import concourse.tile as tile
from concourse import mybir
from concourse.bass_utils import run_bass_kernel_spmd

N_CORES = 8
N, F, D = 16384, 1024, 8
FC = F // N_CORES  # features per core (128)
EPS = 1e-5

F32 = mybir.dt.float32
F16 = mybir.dt.float16
AF = mybir.ActivationFunctionType
OP = mybir.AluOpType

LOAD_CHUNK = 4096
POOL_S1 = False  # Pool rejects the tensor_scalar accum (CACHE_REDUCE) variant


def _split_multiwait_instructions(nc):
    """Walrus codegen encodes at most ONE sync wait per engine instruction.
    Tile may attach several; hoist all but the last into standalone
    InstEventSemaphore instructions on the same engine, placed before."""
    n = 0
    for fn in nc.m.functions:
        for block in fn.blocks:
            out = []
            for inst in block.instructions:
                si = inst.sync_info
                waits = list(si.on_wait) if si is not None else []
                if len(waits) > 1:
                    for w in waits[:-1]:
                        ev = mybir.InstEventSemaphore(
                            name=f"{inst.name}-ws{n}", ins=[], outs=[]
                        )
                        ev.engine = inst.engine
                        ev.sync_info = mybir.SyncInfo(on_wait=[w], on_update=[])
                        out.append(ev)
                        n += 1
                    inst.sync_info = mybir.SyncInfo(
                        on_wait=[waits[-1]], on_update=list(si.on_update)
                    )
                out.append(inst)
            block.instructions = out
    return n


def _ranges(counts):
    """[(start, end, cnt, d)] for domains with cnt > 0, in column order."""
    out = []
    a = 0
    for d, c in enumerate(counts):
        if c > 0:
            out.append((a, a + int(c), int(c), d))
        a += int(c)
    return out


def _schedule_stats(rngs, n_load, S):
    """List-schedule the s1/s2 jobs on DVE/ACT/Pool by data arrival.

    Returns {engine: [(kind, range)]} issue orders. Arrival of a range =
    modeled landing time of its last covering load chunk; engine rates are
    ns/col estimates from the measured v2 trace.
    """
    t_chunk = lambda k: 1.5 + 2.95 * (k + 1)  # modeled chunk-land time (us)
    arrive = {}
    for (a, b, cnt, d) in rngs:
        arrive[d] = t_chunk(min((b - 1) // LOAD_CHUNK, n_load - 1))
    # engine -> (rate ns/col, fixed overhead us)
    lanes = {"dve": 0.00229, "act": 0.00250, "pool": 0.0036}
    free = {"dve": 0.0, "act": 0.0, "pool": 0.0}
    out = {"dve": [], "act": [], "pool": []}
    jobs = []
    for r in rngs:
        d = r[3]
        jobs.append(("s1", r, arrive[d]))
        jobs.append(("s2", r, arrive[d]))
    jobs.sort(key=lambda j: (j[2], j[1][3]))
    for kind, r, t in jobs:
        w = r[1] - r[0]
        cands = []
        for eng in ("dve", "act", "pool"):
            if kind == "s2" and eng != "act" and eng != "dve":
                continue  # s2 only on ACT (square+accum) or DVE (2-pass)
            if kind == "s1" and eng == "act":
                continue  # keep ACT for s2 (it is the only fast square)
            if eng == "pool" and not POOL_S1:
                continue
            rate = lanes[eng] * (1.55 if (kind == "s2" and eng == "dve") else 1.0)
            fin = max(free[eng], t) + w * rate
            cands.append((fin, eng))
        fin, eng = min(cands)
        free[eng] = fin
        out[eng].append((kind, r))
    return out


def _build_kernel(counts):
    S = int(sum(counts))
    W = max(int(c) for c in counts) if len(counts) else 1
    nc = bass.Bass("TRN2", target_bir_lowering=False, debug=False,
                   num_devices=N_CORES)
    x_d = nc.dram_tensor("x", [FC, S], F16, kind="ExternalInput")
    g_d = nc.dram_tensor("gammaT", [FC, D], F32, kind="ExternalInput")
    b_d = nc.dram_tensor("betaT", [FC, D], F32, kind="ExternalInput")
    o_d = nc.dram_tensor("out", [FC, S], F16, kind="ExternalOutput")

    with tile.TileContext(nc) as tc:
        _body(tc, counts, S, W, x_d, g_d, b_d, o_d)
    return nc


def _body(tc, counts, S, W, x_d, g_d, b_d, o_d):
    nc = tc.nc
    from contextlib import ExitStack

    rngs = _ranges(counts)
    n_load = (S + LOAD_CHUNK - 1) // LOAD_CHUNK

    with ExitStack() as ctx:
        big = ctx.enter_context(tc.tile_pool(name="big", bufs=1))
        small = ctx.enter_context(tc.tile_pool(name="small", bufs=1))

        xt = big.tile([FC, S], F16)
        ot = big.tile([FC, S], F16)
        scr_v = big.tile([FC, W], F16)   # DVE scratch (dummy-out / squares)
        scr_a = big.tile([FC, W], F16)   # ACT scratch
        scr_p = big.tile([FC, W], F16)   # Pool scratch

        gt = small.tile([FC, D], F32)
        bt = small.tile([FC, D], F32)
        nc.gpsimd.dma_start(gt[:, :], g_d[:, :])
        nc.gpsimd.dma_start(bt[:, :], b_d[:, :])

        rc8 = small.tile([FC, D], F32)
        for d in range(D):
            nc.vector.memset(rc8[:, d : d + 1], 1.0 / max(int(counts[d]), 1))

        s1 = small.tile([FC, D], F32)
        s2 = small.tile([FC, D], F32)
        for t in (s1, s2):
            nc.vector.memset(t[:, :], 0.0)

        # ---- load x: one chunk per domain (stats deps align exactly with
        # a single chunk), alternating sync + gpsimd rings -----------------
        half_n = (len(rngs) + 1) // 2
        for k, (a, b, cnt, d) in enumerate(rngs):
            # first half on sync; second half issued up-front on the scalar
            # ring (before ACT's first square) so both rings transfer in
            # parallel without touching the slow gpsimd ring
            eng = nc.sync if k < half_n else nc.scalar
            eng.dma_start(xt[:, a:b], x_d[:, a:b])
        tail = rngs[-1][1] if rngs else 0
        if tail < S:  # columns of empty trailing domains (none normally)
            nc.sync.dma_start(xt[:, tail:S], x_d[:, tail:S])

        # ---- phase B in two half-batches, then apply + store -------------
        mean = small.tile([FC, D], F32)
        m2 = small.tile([FC, D], F32)
        var = small.tile([FC, D], F32)
        sd = small.tile([FC, D], F32)
        inv = small.tile([FC, D], F32)
        scale = small.tile([FC, D], F32)
        ms = small.tile([FC, D], F32)
        off = small.tile([FC, D], F32)
        sc16 = small.tile([FC, D], F16)
        of16 = small.tile([FC, D], F16)
        eps_t = small.tile([FC, 1], F32)
        nc.vector.memset(eps_t[:, :], float(EPS))

        def stats(batch):
            for (a, b, cnt, d) in batch:
                w = b - a
                col = slice(d, d + 1)
                nc.vector.tensor_scalar(
                    scr_v[:, 0:w], xt[:, a:b], 1.0, None, OP.mult, OP.add,
                    accum_out=s1[:, col],
                )
                nc.scalar.activation(
                    scr_a[:, 0:w], xt[:, a:b], AF.Square,
                    accum_out=s2[:, col],
                )

        def phase_b(batch):
            lo = min(r[3] for r in batch)
            hi = max(r[3] for r in batch) + 1
            cs = slice(lo, hi)
            # pre-sqrt chain on the idle Pool engine: keeps the ACT sqrt
            # from stalling behind DVE's s1 queue
            nc.gpsimd.tensor_tensor(mean[:, cs], s1[:, cs], rc8[:, cs], OP.mult)
            nc.gpsimd.tensor_tensor(m2[:, cs], mean[:, cs], mean[:, cs], OP.mult)
            nc.gpsimd.tensor_tensor(var[:, cs], s2[:, cs], rc8[:, cs], OP.mult)
            nc.gpsimd.tensor_tensor(var[:, cs], var[:, cs], m2[:, cs], OP.subtract)
            nc.gpsimd.tensor_scalar_max(var[:, cs], var[:, cs], 0.0)
            nc.scalar.activation(sd[:, cs], var[:, cs], AF.Sqrt, bias=eps_t[:, 0:1])
            nc.vector.reciprocal(inv[:, cs], sd[:, cs])
            nc.vector.tensor_tensor(scale[:, cs], inv[:, cs], gt[:, cs], OP.mult)
            nc.vector.tensor_tensor(ms[:, cs], mean[:, cs], scale[:, cs], OP.mult)
            nc.vector.tensor_tensor(off[:, cs], bt[:, cs], ms[:, cs], OP.subtract)
            for d in range(lo, hi):
                if int(counts[d]) == 1:  # passthrough: out = x
                    nc.vector.memset(scale[:, d : d + 1], 1.0)
                    nc.vector.memset(off[:, d : d + 1], 0.0)

        def apply_store(batch):
            for (a, b, cnt, d) in batch:
                sc_col = scale[:, d : d + 1]
                of_col = off[:, d : d + 1]
                if d >= 6:
                    # last two domains on ACT, after its squares
                    nc.scalar.activation(
                        ot[:, a:b], xt[:, a:b], AF.Identity,
                        bias=of_col, scale=sc_col,
                    )
                else:
                    p = a
                    while p < b:  # DVE 2x path needs <=1024-col ops
                        q = min(p + 1024, b)
                        nc.vector.tensor_scalar(
                            ot[:, p:q], xt[:, p:q], sc_col, of_col,
                            OP.mult, OP.add,
                        )
                        p = q
                # keep store issues for DVE-applied domains off the ACT
                # queue (they would block its later squares/applies)
                eng = nc.scalar if d >= 6 else nc.sync
                eng.dma_start(o_d[:, a:b], ot[:, a:b])

        # two-batch emission (best measured): phase B for the first half of
        # the domains is queued between the stats halves, applies/stores for
        # each half follow as soon as their scale/offset are ready
        batch1 = [r for r in rngs if r[3] < D // 2]
        batch2 = [r for r in rngs if r[3] >= D // 2]
        stats(batch1)
        if batch1:
            phase_b(batch1)
        stats(batch2)
        if batch1:
            apply_store(batch1)
        if batch2:
            phase_b(batch2)
            apply_store(batch2)


_NC_CACHE = {}


def _get_nc(counts):
    key = tuple(int(c) for c in counts)
    if key not in _NC_CACHE:
        nc = _build_kernel(key)
        _split_multiwait_instructions(nc)
        _NC_CACHE[key] = nc
    return _NC_CACHE[key]


def _run(inputs, trace=False, **kw):
    x = np.asarray(inputs["x"])
    y = np.asarray(inputs["y"]).astype(np.int64)
    gamma = np.asarray(inputs["gamma"], dtype=np.float32)
    beta = np.asarray(inputs["beta"], dtype=np.float32)
    n, f = x.shape
    d = gamma.shape[0]

    counts = np.bincount(y, minlength=d).astype(np.int64)
    perm = np.argsort(y, kind="stable")
    xs = x[perm].astype(np.float16)  # [N, F] sorted by domain

    nc = _get_nc(counts)
    in_maps = []
    for c in range(N_CORES):
        sl = slice(c * FC, (c + 1) * FC)
        in_maps.append(
            {
                "x": np.ascontiguousarray(xs[:, sl].T),
                "gammaT": np.ascontiguousarray(gamma[:, sl].T),
                "betaT": np.ascontiguousarray(beta[:, sl].T),
            }
        )
    res = run_bass_kernel_spmd(
        nc, in_maps, core_ids=list(range(N_CORES)), trace=trace, **kw
    )
    out_s = np.empty((n, f), dtype=np.float32)
    for c in range(N_CORES):
        sl = slice(c * FC, (c + 1) * FC)
        out_s[:, sl] = res.results[c]["out"].T
    out = np.empty_like(out_s)
    out[perm] = out_s
    return out, res


def kernel(**inputs) -> np.ndarray:
    out, _ = _run(inputs, trace=False)
    return out
